# revision 9
# baseline (speedup 1.0000x reference)
"""Bass/Tile TRN2 kernel for retrieval-KNN MSE distance matrix.

Computes: out = ||t||^2 + ||s@W.T+b||^2 - 2 * t @ (s@W.T+b).T   [N=4096, M=4096]

Sharding (8 cores, output column-parallel, no collectives):
  core c holds s_rep rows [c*512, (c+1)*512) and computes the full-height
  output block out[:, c*512:(c+1)*512].  Per-core work:
    GEMM1: s_projT[1536, 512] = WT.T @ sT         (K-major, 12x12 k/m chunks)
    GEMM2: out_j[128, 512]    = tT_j.T @ s_projT  accumulated over 12 k-chunks
  t_sq comes from gram-block matmuls (tile.T @ tile, diagonal extracted via
  identity mask + tensor_tensor_reduce) and enters as the fp32 ACT bias;
  s_sq comes from a ones-matmul over the squared projection and is folded
  into PSUM with a compensated K=2 matmul (hi+lo bf16 split, scaled -0.5)
  so the ACT copyback (scale=-2, bias=t_sq) produces the final value.

Matmuls run in bf16 (fp32 inputs cast on-chip by DVE); accumulation fp32.
"""

import numpy as np

import concourse.bacc as bacc
import concourse.bass as bass
import concourse.mybir as mybir
import concourse.tile as tile
from concourse.bass_utils import run_bass_kernel_spmd

N = 4096          # t_rep rows
M = 4096          # s_rep rows
D = 1536          # feature dim
NCORES = 8
MC = M // NCORES  # 512: output columns per core
KC = D // 128     # 12:  contraction chunks
NJ = N // 128     # 32:  output row chunks per core

FP32 = mybir.dt.float32
BF16 = mybir.dt.bfloat16
AF = mybir.ActivationFunctionType


def build_nc(variant="full"):
    nc = bacc.Bacc("TRN2", target_bir_lowering=False, num_devices=NCORES)

    t_in = nc.dram_tensor("t", [NJ, KC, 128, 128], FP32, kind="ExternalInput").ap()
    s_in = nc.dram_tensor("s", [KC, 128, MC], FP32, kind="ExternalInput").ap()
    w_in = nc.dram_tensor("w", [KC, 128, D], FP32, kind="ExternalInput").ap()
    b_in = nc.dram_tensor("b", [KC, 128, 1], FP32, kind="ExternalInput").ap()
    id_in = nc.dram_tensor("ident", [128, 128], FP32, kind="ExternalInput").ap()
    out = nc.dram_tensor("out", [NJ, 128, MC], FP32, kind="ExternalOutput").ap()

    with tile.TileContext(nc) as tc:
        with (
            tc.tile_pool(name="const", bufs=1) as const_pool,
            tc.tile_pool(name="sproj", bufs=1) as sproj_pool,
            tc.tile_pool(name="small", bufs=1) as small_pool,
            tc.tile_pool(name="psum_main", bufs=2, space="PSUM") as psum_main,
            tc.tile_pool(name="psum_aux", bufs=2, space="PSUM") as psum_aux,
        ):
            ident = const_pool.tile([128, 128], FP32)
            nc.sync.dma_start(out=ident[:], in_=id_in[:, :])
            ones_col = const_pool.tile([128, 1], BF16)  # lhsT for s_sq row-matmul
            nc.vector.memset(ones_col[:], 1.0)

            # ---- Phase 1: projection s_projT[d, r] + bias, and s_sq ----
            sproj = []  # 12 tiles [128, MC] bf16
            with (
                tc.tile_pool(name="wts", bufs=2) as wt_pool,
                tc.tile_pool(name="wtb", bufs=1) as wtb_pool,
                tc.tile_pool(name="srep", bufs=2) as s_pool,
                tc.tile_pool(name="srepb", bufs=1) as sb_pool,
                tc.tile_pool(name="bias", bufs=1) as b_pool,
                tc.tile_pool(name="sq", bufs=3) as sq_pool,
            ):
                wt_sb = []
                s_sb = []
                b_sb = []
                for k in range(KC):
                    wt = wt_pool.tile([128, D], FP32, name="wt")
                    nc.sync.dma_start(out=wt[:], in_=w_in[k])
                    wtb = wtb_pool.tile([128, D], BF16, name=f"wtb{k}")
                    nc.vector.tensor_copy(wtb[:], wt[:])
                    wt_sb.append(wtb)

                    st = s_pool.tile([128, MC], FP32, name="st")
                    nc.sync.dma_start(out=st[:], in_=s_in[k])
                    stb = sb_pool.tile([128, MC], BF16, name=f"stb{k}")
                    nc.vector.tensor_copy(stb[:], st[:])
                    s_sb.append(stb)

                    bt = b_pool.tile([128, 1], FP32, name=f"bt{k}")
                    nc.sync.dma_start(out=bt[:], in_=b_in[k])
                    b_sb.append(bt)

                psum_sq = psum_aux.tile([1, MC], FP32, name="psum_ssq")
                for j in range(KC):
                    ps = psum_main.tile([128, MC], FP32, name="psum_p1")
                    for k in range(KC):
                        nc.tensor.matmul(
                            ps[:],
                            lhsT=wt_sb[k][:, j * 128:(j + 1) * 128],
                            rhs=s_sb[k][:],
                            start=(k == 0),
                            stop=(k == KC - 1),
                        )
                    sp = sproj_pool.tile([128, MC], BF16, name=f"sproj{j}")
                    nc.scalar.activation(sp[:], ps[:], AF.Identity,
                                         bias=b_sb[j][:], scale=1.0)
                    sproj.append(sp)
                    # squared projection -> s_sq partial via ones-matmul
                    sq = sq_pool.tile([128, MC], BF16, name="sq")
                    nc.vector.tensor_mul(sq[:], sp[:], sp[:])
                    nc.tensor.matmul(
                        psum_sq[:],
                        lhsT=ones_col[:],
                        rhs=sq[:],
                        start=(j == 0),
                        stop=(j == KC - 1),
                    )

                # s_sq broadcast tile [128, MC] fp32 via log2-doubling DMAs
                ssq_bc = small_pool.tile([128, MC], FP32, name="ssq_bc")
                nc.scalar.activation(ssq_bc[0:1, :], psum_sq[:], AF.Identity)
                sh = 1
                while sh < 128:
                    nc.sync.dma_start(out=ssq_bc[sh:2 * sh, :],
                                      in_=ssq_bc[0:sh, :])
                    sh *= 2

            # ---- Phase 2: main GEMM over 32 row-chunks ----
            with (
                tc.tile_pool(name="tt", bufs=2 * KC) as t_pool,
                tc.tile_pool(name="ttb", bufs=2 * KC) as tb_pool,
                tc.tile_pool(name="osb", bufs=3) as out_pool,
                tc.tile_pool(name="tsq", bufs=3) as tsq_pool,
                tc.tile_pool(name="psum_gram", bufs=2, space="PSUM") as psum_gram,
            ):
                for j in range(NJ):
                    t_sb = []
                    for k in range(KC):
                        tt = t_pool.tile([128, 128], FP32, name="tt")
                        nc.sync.dma_start(out=tt[:], in_=t_in[j, k])
                        ttb = tb_pool.tile([128, 128], BF16, name="ttb")
                        nc.vector.tensor_copy(ttb[:], tt[:])
                        t_sb.append(ttb)

                    ps = psum_main.tile([128, MC], FP32, name="psum_main")
                    use_gram = variant in ("full", "gram", "gram_only", "gram_ttr")
                    use_ttr = variant in ("full", "gram", "gram_ttr")
                    use_bias = variant in ("full", "gram")
                    use_ssq = variant in ("full", "full_nogram")
                    for k in range(KC):
                        nc.tensor.matmul(
                            ps[:],
                            lhsT=t_sb[k][:],
                            rhs=sproj[k][:],
                            start=(k == 0),
                            stop=(k == KC - 1),
                        )
                        if use_gram:
                            if k == 0:
                                gram = psum_gram.tile([128, 128], FP32, name="psum_gram")
                            nc.tensor.matmul(
                                gram[:],
                                lhsT=t_sb[k][:],
                                rhs=t_sb[k][:],
                                start=(k == 0),
                                stop=(k == KC - 1),
                            )
                    ob = out_pool.tile([128, MC], FP32, name="osb")
                    if use_gram and not use_ttr:
                        # consume gram so it isn't dead: copy into scratch and DMA a row out
                        gsb = tsq_pool.tile([128, 128], FP32, name="gsb")
                        nc.scalar.activation(gsb[:], gram[:], AF.Identity)
                        nc.sync.dma_start(out=out[j][:, 0:128], in_=gsb[:])
                    if use_gram and use_ttr:
                        # t_sq[p] = sum_f gram[p, f] * I[p, f]
                        tsq = tsq_pool.tile([128, 1], FP32, name="tsq")
                        scratch = tsq_pool.tile([128, 128], FP32, name="tsq_scratch")
                        nc.vector.tensor_mul(scratch[:], gram[:], ident[:])
                        nc.vector.reduce_sum(tsq[:], scratch[:],
                                             axis=mybir.AxisListType.X)
                        # out = (-2 * cross + t_sq) + s_sq
                        obt = out_pool.tile([128, MC], FP32, name="obt")
                        nc.scalar.activation(obt[:], ps[:], AF.Identity,
                                             bias=tsq[:], scale=-2.0)
                        nc.vector.tensor_add(ob[:], obt[:], ssq_bc[:])
                    else:
                        nc.scalar.activation(ob[:], ps[:], AF.Identity,
                                             scale=-2.0)
                    nc.sync.dma_start(out=out[j], in_=ob[:])

    nc.compile()
    return nc


_NC_CACHE = None


def _get_nc():
    global _NC_CACHE
    if _NC_CACHE is None:
        _NC_CACHE = build_nc()
    return _NC_CACHE


def stage_inputs(t_rep, s_rep, W, b):
    """Host-side layout staging (transpose/tile only) -> per-core input maps."""
    t_rep = np.asarray(t_rep, dtype=np.float32)
    s_rep = np.asarray(s_rep, dtype=np.float32)
    W = np.asarray(W, dtype=np.float32)
    b = np.asarray(b, dtype=np.float32)

    # t tiles: [NJ, KC, 128(d), 128(row)]; tile[j,k][p,c] = t_rep[j*128+c, k*128+p]
    t_tiles = np.ascontiguousarray(
        t_rep.reshape(NJ, 128, KC, 128).transpose(0, 2, 3, 1)
    )
    # WT: [KC, 128, D]; WT[k][p, m] = W[m, k*128+p]
    wt = np.ascontiguousarray(W.T).reshape(KC, 128, D)
    b_st = np.ascontiguousarray(b.reshape(KC, 128, 1))

    in_maps = []
    for c in range(NCORES):
        s_slice = s_rep[c * MC:(c + 1) * MC]  # [512, D]
        # sT: [KC, 128, MC]; sT[k][p, r] = s_slice[r, k*128+p]
        s_st = np.ascontiguousarray(
            s_slice.reshape(MC, KC, 128).transpose(1, 2, 0)
        )
        in_maps.append({"t": t_tiles, "s": s_st, "w": wt, "b": b_st,
                        "ident": np.eye(128, dtype=np.float32)})
    return in_maps


def run_spmd(in_maps, **kwargs):
    nc = _get_nc()
    return run_bass_kernel_spmd(nc, in_maps, core_ids=list(range(NCORES)), **kwargs)


def gather_output(results):
    return np.concatenate(
        [results[c]["out"].reshape(N, MC) for c in range(NCORES)], axis=1
    )


def kernel(t_rep, s_rep, W, b):
    in_maps = stage_inputs(t_rep, s_rep, W, b)
    res = run_spmd(in_maps)
    return gather_output(res.results)
